# revision 1
# baseline (speedup 1.0000x reference)
"""Distance-correlation (DcorLoss) kernel for 8 trn2 NeuronCores.

Math: for x, y [n=8192, d=128]:
  a = pairwise_dist(x), b = pairwise_dist(y)   (n x n, symmetric, zero diag)
  A = double_center(a), B = double_center(b)
  dcor = -sqrt(sum(A*B)) / sqrt(sqrt(sum(A*A)) * sqrt(sum(B*B)))   (n^2 factors cancel)

Key identities (never materialize A/B):
  sum(HaH o HbH) = sum(at o bt) - 2/n * dot(rs_at, rs_bt) + sum(at)*sum(bt)/n^2
for at = a - mu (any constant shift; double centering annihilates it). The
mu ~ E[dist] shift keeps fp32 device accumulations well-conditioned. And the
squared-distance Frobenius norms have a closed form -- no elementwise pass:
  sum_ij dist^2_ij = 2n * sum_i |x_i|^2 - 2 |sum_i x_i|^2
so only sum (a-mu)*b needs streaming; sum(a-mu)^2 / sum(b-mu)^2 come from
row sums + norms + the column-sum vector of x. Cross-core combining is fp64
on host (the only inter-core step; partials are tiny).

Sharding: block-rows; core c owns rows [c*1024, (c+1)*1024), streams all columns.

Per (128-row x 1024-col) tile pair, the device computes:
  PE:   psum = -2*x_blk^T x (K=128, bf16) + ones2 (x) [n_hi; n_lo] (K=2 bf16
        hi/lo split of the fp32 column norms -> ~16-bit mantissa)
  DVE:  psum[diag block] += mu^2 * I  (data-driven: per-core `diagsel` input
        is nonzero only on the core's own diagonal window)
  ACT:  t = sqrt(psum + n_i)  [per-partition fp32 bias], accum_out -> row sums
  DVE:  (t_a - mu) * t_b -> accum_out   (one scalar_tensor_tensor)
Forcing the diagonal of sq to mu^2 keeps sqrt NaN-free; host replaces the known
diagonal contribution exactly (true diag of a is 0).
"""

import os

import numpy as np

import concourse.bass as bass
import concourse.tile as tile
from concourse import bacc, mybir
from concourse.bass_utils import run_bass_kernel_spmd

P = 128            # partitions / d
N = 8192           # points
NCORES = 8
BLK = N // NCORES  # 1024 rows per core
CI_N = BLK // P    # 8 row chunks per core
W = 1024           # column window
JT_N = N // W      # 8 column windows
MU = 16.0          # ~E[pairwise dist] for randn d=128; any constant is exact math
MU2 = MU * MU
RES_W = 48

_programs = {}


def _build(mm_mode: str):
    """mm_mode: 'bf16' | 'f32' (matmul operand dtype)."""
    dt = mybir.dt
    f32 = dt.float32
    mmdt = dt.bfloat16 if mm_mode == "bf16" else dt.float32
    A = mybir.AluOpType
    AF = mybir.ActivationFunctionType

    nc = bacc.Bacc("TRN2", target_bir_lowering=False, debug=False,
                   num_devices=NCORES)

    dxT = nc.dram_tensor("xT", [P, N], f32, kind="ExternalInput").ap()
    dyT = nc.dram_tensor("yT", [P, N], f32, kind="ExternalInput").ap()
    dxb = nc.dram_tensor("xblkT", [P, BLK], f32, kind="ExternalInput").ap()
    dyb = nc.dram_tensor("yblkT", [P, BLK], f32, kind="ExternalInput").ap()
    ddg = nc.dram_tensor("diagsel", [P, JT_N * P], f32, kind="ExternalInput").ap()
    dew = nc.dram_tensor("eyewide", [P, 4 * 512], f32, kind="ExternalInput").ap()
    dout = nc.dram_tensor("out", [P, RES_W], f32, kind="ExternalOutput").ap()

    with tile.TileContext(nc) as tc:
        with tc.tile_pool(name="const", bufs=1) as cp, \
             tc.tile_pool(name="psum", bufs=1, space="PSUM") as pp, \
             tc.tile_pool(name="ab", bufs=3) as abp, \
             tc.tile_pool(name="trd", bufs=2) as trd:

            # ── persistent operands ────────────────────────────────────
            xTc = cp.tile([P, N], mmdt, tag="xTc")
            yTc = cp.tile([P, N], mmdt, tag="yTc")
            xblk2 = cp.tile([P, BLK], mmdt, tag="xblk2")
            yblk2 = cp.tile([P, BLK], mmdt, tag="yblk2")
            # column norms as bf16 hi/lo rows: n_j = hi_j + lo_j (K=2 matmul)
            nfhl_x = cp.tile([2, N], mmdt, tag="nfhl_x")
            nfhl_y = cp.tile([2, N], mmdt, tag="nfhl_y")
            nbx = cp.tile([P, CI_N], f32, tag="nbx")
            nby = cp.tile([P, CI_N], f32, tag="nby")
            diag_m = cp.tile([P, JT_N * P], mmdt, tag="diag_m")
            eyew_m = cp.tile([P, 4 * 512], mmdt, tag="eyew_m")
            ones2 = cp.tile([2, P], mmdt, tag="ones2")
            nc.vector.memset(ones2[:], 1.0)
            onesc = cp.tile([P, 1], mmdt, tag="onesc")
            nc.vector.memset(onesc[:], 1.0)
            quarc = cp.tile([P, 1], f32, tag="quarc")
            nc.vector.memset(quarc[:], 0.25)

            res = cp.tile([P, RES_W], f32, tag="res")
            nc.vector.memset(res[:], 0.0)

            # PE warm-up: ~5us of dense back-to-back matmuls on constant data
            # so the HAM clock-gate reaches 8/8 before real work starts (cold
            # first executions otherwise run the whole kernel at half clock)
            wur = cp.tile([2, 512], mmdt, tag="wur")
            nc.vector.memset(wur[:], 0.0)
            wt = pp.tile([P, W], f32, tag="a", bufs=2)
            for _ in range(24):
                nc.tensor.matmul(wt[:, 0:512], ones2[:], wur[:],
                                 start=True, stop=True)

            # ── setup (scratch pool closes before the main loop) ──────
            with tc.tile_pool(name="setup", bufs=1) as sp:
                big = sp.tile([P, N], f32, tag="big")
                nc.sync.dma_start(big[:], dxT[:])
                nc.scalar.copy(xTc[:], big[:])
                big2 = sp.tile([P, N], f32, tag="big")
                nc.sync.dma_start(big2[:], dyT[:])
                nc.scalar.copy(yTc[:], big2[:])

                for dsrc, dst in ((dxb, xblk2), (dyb, yblk2)):
                    raw = sp.tile([P, BLK], f32, tag="braw")
                    nc.sync.dma_start(raw[:], dsrc[:])
                    nc.scalar.mul(dst[:], raw[:], -2.0)

                dgr = sp.tile([P, JT_N * P], f32, tag="dgr")
                nc.sync.dma_start(dgr[:], ddg[:])
                nc.scalar.copy(diag_m[:], dgr[:])
                ewr = sp.tile([P, 4 * 512], f32, tag="ewr")
                nc.sync.dma_start(ewr[:], dew[:])
                nc.scalar.copy(eyew_m[:], ewr[:])

                # full column norms nf[j] = sum_d xTc[d, j]^2 (fp32 in PSUM),
                # split per-slice into bf16 hi/lo; DMA does the partition
                # placement into nfhl (engines are lane-aligned)
                for src, nfhl in ((xTc, nfhl_x), (yTc, nfhl_y)):
                    hi = sp.tile([1, N], mmdt, tag="hi")
                    lo = sp.tile([1, N], mmdt, tag="lo")
                    for j8 in range(JT_N):
                        sq = sp.tile([P, W], mmdt, tag="sq", bufs=2)
                        nc.scalar.activation(sq[:], src[:, bass.ts(j8, W)],
                                             AF.Square)
                        for h in range(2):
                            ps = pp.tile([P, W], f32, tag="a", bufs=2)
                            sl = bass.ds(j8 * W + h * 512, 512)
                            nc.tensor.matmul(ps[0:1, 0:512], onesc[:],
                                             sq[:, bass.ts(h, 512)],
                                             start=True, stop=True)
                            nc.vector.tensor_copy(hi[0:1, sl],
                                                  ps[0:1, 0:512])
                            nc.vector.tensor_tensor(lo[0:1, sl],
                                                    ps[0:1, 0:512],
                                                    hi[0:1, sl],
                                                    op=A.subtract)
                        # ship this window now so main-loop iterations can
                        # start before the whole norms row is built
                        wsl = bass.ts(j8, W)
                        nc.sync.dma_start(nfhl[0:1, wsl], hi[0:1, wsl])
                        nc.sync.dma_start(nfhl[1:2, wsl], lo[0:1, wsl])

                # block norms as [P, CI_N] fp32 columns (sqrt bias):
                # (-2x)^2 * 0.25 = x^2
                for src, dst in ((xblk2, nbx), (yblk2, nby)):
                    sqb = sp.tile([P, BLK], f32, tag="sqb")
                    nc.scalar.activation(sqb[:], src[:], AF.Square)
                    for ci in range(CI_N):
                        ps = pp.tile([P, W], f32, tag="a", bufs=2)
                        nc.tensor.matmul(ps[:, 0:1], sqb[:, bass.ts(ci, P)],
                                         quarc[:], start=True, stop=True)
                        nc.vector.tensor_copy(dst[:, ci:ci + 1], ps[:, 0:1])

            # closed-form helpers: sum of norms (hi+lo rows reduced) and the
            # column-sum vector s = sum_i x_i, both over the bf16 values
            nc.vector.tensor_reduce(res[0:2, 41:42], nfhl_x[:, :],
                                    axis=mybir.AxisListType.X, op=A.add)
            nc.vector.tensor_reduce(res[0:2, 42:43], nfhl_y[:, :],
                                    axis=mybir.AxisListType.X, op=A.add)
            nc.vector.tensor_reduce(res[:, 44:45], xTc[:, :],
                                    axis=mybir.AxisListType.X, op=A.add)
            nc.vector.tensor_reduce(res[:, 45:46], yTc[:, :],
                                    axis=mybir.AxisListType.X, op=A.add)

            # ── stages ────────────────────────────────────────────────
            st = [cp.tile([P, CI_N * JT_N], f32, tag=f"st{q}", name=f"st{q}")
                  for q in range(3)]

            # ── main loop ─────────────────────────────────────────────
            for ci in range(CI_N):
                for jt in range(JT_N):
                    col = ci * JT_N + jt
                    h_diag = ci // 4
                    psA = pp.tile([P, W], f32, tag="a", bufs=2)
                    psB = pp.tile([P, W], f32, tag="b", bufs=2)
                    # weight-grouped order: mains (xblk2 / yblk2), then all
                    # norm matmuls (shared ones2 weights), then the diag eye
                    for ps_, blk2, full in ((psA, xblk2, xTc),
                                            (psB, yblk2, yTc)):
                        for h in range(2):
                            nc.tensor.matmul(
                                ps_[:, bass.ds(h * 512, 512)],
                                blk2[:, bass.ts(ci, P)],
                                full[:, bass.ds(jt * W + h * 512, 512)],
                                start=True, stop=False)
                    for ps_, nfhl in ((psA, nfhl_x), (psB, nfhl_y)):
                        for h in range(2):
                            nc.tensor.matmul(
                                ps_[:, bass.ds(h * 512, 512)], ones2[:],
                                nfhl[:, bass.ds(jt * W + h * 512, 512)],
                                start=False, stop=(h != h_diag))
                    for ps_ in (psA, psB):
                        # += mu^2*I on the diag window (zeros unless jt ==
                        # core id): (mu I)^T (mu I @ offset)
                        nc.tensor.matmul(ps_[:, bass.ds(h_diag * 512, 512)],
                                         diag_m[:, bass.ts(jt, P)],
                                         eyew_m[:, bass.ts(ci % 4, 512)],
                                         start=False, stop=True)

                    aT = abp.tile([P, W], f32, tag="a")
                    bT = abp.tile([P, W], f32, tag="b")
                    nc.scalar.activation(aT[:], psA[:], AF.Sqrt,
                                         bias=nbx[:, ci:ci + 1],
                                         accum_out=st[0][:, col:col + 1])
                    nc.scalar.activation(bT[:], psB[:], AF.Sqrt,
                                         bias=nby[:, ci:ci + 1],
                                         accum_out=st[1][:, col:col + 1])
                    t0 = trd.tile([P, W], f32, tag="t")
                    nc.vector.scalar_tensor_tensor(
                        t0[:], aT[:], MU, bT[:], op0=A.subtract, op1=A.mult,
                        accum_out=st[2][:, col:col + 1])

            # ── epilogue ──────────────────────────────────────────────
            nc.vector.tensor_copy(res[:, 24:24 + CI_N], nbx[:, :])
            nc.vector.tensor_copy(res[:, 32:32 + CI_N], nby[:, :])
            for q in range(3):
                for ci in range(CI_N):
                    o = q * CI_N + ci
                    nc.vector.tensor_reduce(res[:, o:o + 1],
                                            st[q][:, bass.ts(ci, JT_N)],
                                            axis=mybir.AxisListType.X,
                                            op=A.add)
            nc.sync.dma_start(dout[:], res[:])

    nc.compile()
    return nc


def _get_program(mm_mode: str):
    if mm_mode not in _programs:
        _programs[mm_mode] = _build(mm_mode)
    return _programs[mm_mode]


def make_in_maps(x: np.ndarray, y: np.ndarray):
    x = np.ascontiguousarray(np.asarray(x, np.float32))
    y = np.ascontiguousarray(np.asarray(y, np.float32))
    xT = np.ascontiguousarray(x.T)
    yT = np.ascontiguousarray(y.T)
    eye = (np.eye(P, dtype=np.float32) * MU)
    ew = np.zeros((P, 4 * 512), np.float32)
    for k in range(4):
        for p in range(P):
            ew[p, k * 512 + k * P + p] = MU
    in_maps = []
    for c in range(NCORES):
        dg = np.zeros((P, JT_N * P), np.float32)
        dg[:, c * P:(c + 1) * P] = eye
        in_maps.append({
            "xT": xT,
            "yT": yT,
            "xblkT": np.ascontiguousarray(x[c * BLK:(c + 1) * BLK].T),
            "yblkT": np.ascontiguousarray(y[c * BLK:(c + 1) * BLK].T),
            "diagsel": dg,
            "eyewide": ew,
        })
    return in_maps


def finalize(outs):
    """outs: list of 8 [128, 48] arrays -> scalar dcor (fp64 host math).

    Cols: rsa 0:8 | rsb 8:16 | pab 16:24 | [0:2,41]=(sum hi, sum lo) of x
    norms | [0:2,42]= same for y | [:,44]=sum_i x_i | [:,45]=sum_i y_i.
    Device row sums include the forced diag ~mu (true diag of a is 0).
    """
    n = float(N)
    rs_a = np.empty(N, np.float64)
    rs_b = np.empty(N, np.float64)
    pab = 0.0
    for c, o in enumerate(outs):
        o = np.asarray(o, np.float64)
        rs_a[c * BLK:(c + 1) * BLK] = o[:, 0:CI_N].T.ravel()
        rs_b[c * BLK:(c + 1) * BLK] = o[:, CI_N:2 * CI_N].T.ravel()
        pab += o[:, 2 * CI_N:3 * CI_N].sum()

    o0 = np.asarray(outs[0], np.float64)
    # column-norm sums as the device's K=2 matmul sees them (bf16 hi+lo of
    # bf16-rounded squares); row-bias norms are the fp32-exact path
    sum_nxc = o0[0, 41] + o0[1, 41]
    sum_nyc = o0[0, 42] + o0[1, 42]
    sum_nxr = sum(np.asarray(o, np.float64)[:, 24:24 + CI_N].sum()
                  for o in outs)
    sum_nyr = sum(np.asarray(o, np.float64)[:, 32:32 + CI_N].sum()
                  for o in outs)
    sx = o0[:, 44]                        # sum_i x_i  [128]
    sy = o0[:, 45]
    # closed-form squared-distance Frobenius norms, consistent with the
    # device's mixed n_i/n_j paths (true zero diag):
    sq_a = n * (sum_nxr + sum_nxc) - 2.0 * np.dot(sx, sx)   # sum_ij a_ij^2
    sq_b = n * (sum_nyr + sum_nyc) - 2.0 * np.dot(sy, sy)

    sa = rs_a - MU          # true (zero-diag) row sums of a
    sb = rs_b - MU
    sat = sa - n * MU       # row sums of (a - mu)
    sbt = sb - n * MU
    Ua = sat.sum()
    Ub = sbt.sum()
    # device pab = sum (a-mu)*b (diag contributes ~0 in device and truth)
    Sab = pab - MU * (sa.sum() - MU * n * n)
    Saa = sq_a - 2.0 * MU * sa.sum() + MU2 * n * n
    Sbb = sq_b - 2.0 * MU * sb.sum() + MU2 * n * n

    sumAB = Sab - 2.0 * np.dot(sat, sbt) / n + Ua * Ub / n**2
    sumAA = Saa - 2.0 * np.dot(sat, sat) / n + Ua * Ua / n**2
    sumBB = Sbb - 2.0 * np.dot(sbt, sbt) / n + Ub * Ub / n**2

    inv_n2 = 1.0 / (n * n)
    dcov2_xy = sumAB * inv_n2
    dcov2_xx = sumAA * inv_n2
    dcov2_yy = sumBB * inv_n2
    dcor = -np.sqrt(dcov2_xy) / np.sqrt(np.sqrt(dcov2_xx) * np.sqrt(dcov2_yy))
    return np.asarray(dcor, dtype=np.float32)


def run(x, y, mm_mode=None, trace=False, tmpdir=None):
    if mm_mode is None:
        mm_mode = os.environ.get("DCOR_MM", "bf16")
    nc = _get_program(mm_mode)
    in_maps = make_in_maps(x, y)
    res = run_bass_kernel_spmd(nc, in_maps, core_ids=list(range(NCORES)),
                               trace=trace, tmpdir=tmpdir)
    outs = [r["out"] for r in res.results]
    return finalize(outs), res


def kernel(x, y):
    val, _ = run(x, y)
    return val



# revision 2
# speedup vs baseline: 1.0087x; 1.0087x over previous
"""Distance-correlation (DcorLoss) kernel for 8 trn2 NeuronCores.

Math: for x, y [n=8192, d=128]:
  a = pairwise_dist(x), b = pairwise_dist(y)   (n x n, symmetric, zero diag)
  A = double_center(a), B = double_center(b)
  dcor = -sqrt(sum(A*B)) / sqrt(sqrt(sum(A*A)) * sqrt(sum(B*B)))

Key identities (never materialize A/B):
  sum(HaH o HbH) = sum(at o bt) - 2/n * dot(rs_at, rs_bt) + sum(at)*sum(bt)/n^2
for at = a - mu (double centering annihilates the constant shift), and the
squared-distance Frobenius norms have a closed form from norms + column sums,
so only row sums of a/b and sum (a-mu)*b need streaming.

Device work per (128-row x 1024-col) tile pair (64 tiles per core):
  PE:   psum = -2*x_blk^T x (K=128, bf16) + ones2 (x) [n_hi; n_lo] (K=2)
  ACT:  t = sqrt(psum + n_i)  [per-partition fp32 bias], accum_out -> row sums
  DVE:  (t_a - mu) * t_b -> accum_out
All norms / scaled operands are precomputed on HOST and shipped as bf16/f32
inputs (no on-device setup passes). Each core's columns are ROTATED by
c*1024 so its diagonal block always sits in local window 0: the mu^2*I
diagonal fix runs only for jt==0 (8 matmuls/psum instead of 64).

Sharding: block-rows; core c owns rows [c*1024, (c+1)*1024), streams all
columns (in rotated order; all reductions are column-order invariant).
Cross-core combining is fp64 on host.
"""

import numpy as np
import ml_dtypes

import concourse.bass as bass
import concourse.tile as tile
from concourse import bacc, mybir
from concourse.bass_utils import run_bass_kernel_spmd

P = 128            # partitions / d
N = 8192           # points
NCORES = 8
BLK = N // NCORES  # 1024 rows per core
CI_N = BLK // P    # 8 row chunks per core
W = 1024           # column window
JT_N = N // W      # 8 column windows
MU = 16.0          # ~E[pairwise dist] for randn d=128; any constant is exact
MU2 = MU * MU
RES_W = 24
BF = ml_dtypes.bfloat16

_programs = {}


def _build():
    dt = mybir.dt
    f32 = dt.float32
    bf16 = dt.bfloat16
    A = mybir.AluOpType
    AF = mybir.ActivationFunctionType

    nc = bacc.Bacc("TRN2", target_bir_lowering=False, debug=False,
                   num_devices=NCORES)

    dxT = nc.dram_tensor("xT", [P, N], bf16, kind="ExternalInput").ap()
    dyT = nc.dram_tensor("yT", [P, N], bf16, kind="ExternalInput").ap()
    dxb = nc.dram_tensor("xblkT", [P, BLK], bf16, kind="ExternalInput").ap()
    dyb = nc.dram_tensor("yblkT", [P, BLK], bf16, kind="ExternalInput").ap()
    dnfx = nc.dram_tensor("nfx", [2, N], bf16, kind="ExternalInput").ap()
    dnfy = nc.dram_tensor("nfy", [2, N], bf16, kind="ExternalInput").ap()
    dnbx = nc.dram_tensor("nbx", [P, CI_N], f32, kind="ExternalInput").ap()
    dnby = nc.dram_tensor("nby", [P, CI_N], f32, kind="ExternalInput").ap()
    ddg = nc.dram_tensor("diagm", [P, P], bf16, kind="ExternalInput").ap()
    dew = nc.dram_tensor("eyewide", [P, 4 * 512], bf16,
                         kind="ExternalInput").ap()
    dout = nc.dram_tensor("out", [P, RES_W], f32, kind="ExternalOutput").ap()

    with tile.TileContext(nc) as tc:
        with tc.tile_pool(name="const", bufs=1) as cp, \
             tc.tile_pool(name="psum", bufs=1, space="PSUM") as pp, \
             tc.tile_pool(name="ab", bufs=3) as abp, \
             tc.tile_pool(name="trd", bufs=2) as trd:

            # ── persistent operands (DMA'd straight in, no conversion) ──
            xTc = cp.tile([P, N], bf16, tag="xTc")
            yTc = cp.tile([P, N], bf16, tag="yTc")
            xblk2 = cp.tile([P, BLK], bf16, tag="xblk2")
            yblk2 = cp.tile([P, BLK], bf16, tag="yblk2")
            nfhl_x = cp.tile([2, N], bf16, tag="nfhl_x")
            nfhl_y = cp.tile([2, N], bf16, tag="nfhl_y")
            nbx = cp.tile([P, CI_N], f32, tag="nbx")
            nby = cp.tile([P, CI_N], f32, tag="nby")
            diag_m = cp.tile([P, P], bf16, tag="diag_m")
            eyew_m = cp.tile([P, 4 * 512], bf16, tag="eyew_m")
            ones2 = cp.tile([2, P], bf16, tag="ones2")
            nc.vector.memset(ones2[:], 1.0)

            # input DMAs (chunked for queue parallelism)
            for h in range(4):
                sl = bass.ts(h, N // 4)
                nc.sync.dma_start(xTc[:, sl], dxT[:, sl])
                nc.sync.dma_start(yTc[:, sl], dyT[:, sl])
            nc.sync.dma_start(xblk2[:], dxb[:])
            nc.sync.dma_start(yblk2[:], dyb[:])
            nc.sync.dma_start(nfhl_x[:], dnfx[:])
            nc.sync.dma_start(nfhl_y[:], dnfy[:])
            nc.sync.dma_start(nbx[:], dnbx[:])
            nc.sync.dma_start(nby[:], dnby[:])
            nc.sync.dma_start(diag_m[:], ddg[:])
            nc.sync.dma_start(eyew_m[:], dew[:])

            # PE warm-up: dense back-to-back matmuls on constant data so the
            # clock-gate ramps to full before real work (overlaps the DMAs)
            wur = cp.tile([2, 512], bf16, tag="wur")
            nc.vector.memset(wur[:], 0.0)
            wt = pp.tile([P, W], f32, tag="a", bufs=2)
            for _ in range(24):
                nc.tensor.matmul(wt[:, 0:512], ones2[:], wur[:],
                                 start=True, stop=True)

            # ── stages ────────────────────────────────────────────────
            st = [cp.tile([P, CI_N * JT_N], f32, tag=f"st{q}", name=f"st{q}")
                  for q in range(3)]

            # ── main loop ─────────────────────────────────────────────
            for ci in range(CI_N):
                for jt in range(JT_N):
                    col = ci * JT_N + jt
                    h_diag = ci // 4
                    psA = pp.tile([P, W], f32, tag="a", bufs=2)
                    psB = pp.tile([P, W], f32, tag="b", bufs=2)
                    for ps_, blk2, full in ((psA, xblk2, xTc),
                                            (psB, yblk2, yTc)):
                        for h in range(2):
                            nc.tensor.matmul(
                                ps_[:, bass.ds(h * 512, 512)],
                                blk2[:, bass.ts(ci, P)],
                                full[:, bass.ds(jt * W + h * 512, 512)],
                                start=True, stop=False)
                    for ps_, nfhl in ((psA, nfhl_x), (psB, nfhl_y)):
                        for h in range(2):
                            nc.tensor.matmul(
                                ps_[:, bass.ds(h * 512, 512)], ones2[:],
                                nfhl[:, bass.ds(jt * W + h * 512, 512)],
                                start=False,
                                stop=(jt != 0 or h != h_diag))
                    if jt == 0:
                        # diag window (rotation puts it at jt==0 on every
                        # core): += mu^2*I at block offset ci*128
                        for ps_ in (psA, psB):
                            nc.tensor.matmul(
                                ps_[:, bass.ds(h_diag * 512, 512)],
                                diag_m[:, 0:P],
                                eyew_m[:, bass.ts(ci % 4, 512)],
                                start=False, stop=True)

                    aT = abp.tile([P, W], f32, tag="a")
                    bT = abp.tile([P, W], f32, tag="b")
                    nc.scalar.activation(aT[:], psA[:], AF.Sqrt,
                                         bias=nbx[:, ci:ci + 1],
                                         accum_out=st[0][:, col:col + 1])
                    nc.scalar.activation(bT[:], psB[:], AF.Sqrt,
                                         bias=nby[:, ci:ci + 1],
                                         accum_out=st[1][:, col:col + 1])
                    t0 = trd.tile([P, W], f32, tag="t")
                    nc.vector.scalar_tensor_tensor(
                        t0[:], aT[:], MU, bT[:], op0=A.subtract, op1=A.mult,
                        accum_out=st[2][:, col:col + 1])

            # ── epilogue: reduce st -> res, ship out ──────────────────
            res = cp.tile([P, RES_W], f32, tag="res")
            for q in range(3):
                for ci in range(CI_N):
                    o = q * CI_N + ci
                    nc.vector.tensor_reduce(res[:, o:o + 1],
                                            st[q][:, bass.ts(ci, JT_N)],
                                            axis=mybir.AxisListType.X,
                                            op=A.add)
            nc.sync.dma_start(dout[:], res[:])

    nc.compile()
    return nc


def _get_program():
    if "main" not in _programs:
        _programs["main"] = _build()
    return _programs["main"]


def make_in_maps(x: np.ndarray, y: np.ndarray):
    """Host-precomputed, per-core-rotated bf16 inputs + fp64 aux sums."""
    x = np.ascontiguousarray(np.asarray(x, np.float32))
    y = np.ascontiguousarray(np.asarray(y, np.float32))

    aux = {}
    in_maps = [dict() for _ in range(NCORES)]
    for name, v in (("x", x), ("y", y)):
        vb = v.astype(BF)                      # bf16 of x (matmul stream)
        vT = np.ascontiguousarray(vb.T)        # [128, N] bf16
        vf = vT.astype(np.float32)
        nf32 = (vf * vf).sum(axis=0, dtype=np.float32)      # col norms [N]
        hi = nf32.astype(BF)
        lo = (nf32 - hi.astype(np.float32)).astype(BF)
        vm2 = (-2.0 * v).astype(BF)            # bf16(-2x) (stationary)
        vm2f = vm2.astype(np.float32)
        nb_full = 0.25 * (vm2f * vm2f).sum(axis=1, dtype=np.float32)  # [N]
        aux["sum_n" + name] = (nf32.astype(np.float64).sum()
                               + np.float64(0.0))  # col-norm path
        aux["sum_nb" + name] = nb_full.astype(np.float64).sum()
        aux["s" + name] = vf.astype(np.float64).sum(axis=1)  # [128]
        for c in range(NCORES):
            rl = np.roll(vT, -c * BLK, axis=1)
            nfr = np.stack([np.roll(hi, -c * BLK), np.roll(lo, -c * BLK)])
            blkT = np.ascontiguousarray(vm2[c * BLK:(c + 1) * BLK].T)
            nb = np.ascontiguousarray(
                nb_full[c * BLK:(c + 1) * BLK].reshape(CI_N, P).T)
            pre = "x" if name == "x" else "y"
            in_maps[c][pre + "T"] = np.ascontiguousarray(rl)
            in_maps[c][pre + "blkT"] = blkT
            in_maps[c]["nf" + pre] = np.ascontiguousarray(nfr)
            in_maps[c]["nb" + pre] = nb.astype(np.float32)

    diagm = (np.eye(P, dtype=np.float32) * MU).astype(BF)
    ew = np.zeros((P, 4 * 512), np.float32)
    for k in range(4):
        for p in range(P):
            ew[p, k * 512 + k * P + p] = MU
    ewb = ew.astype(BF)
    for c in range(NCORES):
        in_maps[c]["diagm"] = diagm
        in_maps[c]["eyewide"] = ewb
    return in_maps, aux


def finalize(outs, aux):
    """outs: 8 x [128, 24] arrays -> scalar dcor (fp64 host math).

    Cols: rsa 0:8 | rsb 8:16 | pab 16:24. Device row sums include the
    forced diag ~mu (true diag of a is 0). All norm/column sums come from
    host aux (computed from the exact bf16 arrays shipped to the device).
    """
    n = float(N)
    rs_a = np.empty(N, np.float64)
    rs_b = np.empty(N, np.float64)
    pab = 0.0
    for c, o in enumerate(outs):
        o = np.asarray(o, np.float64)
        rs_a[c * BLK:(c + 1) * BLK] = o[:, 0:CI_N].T.ravel()
        rs_b[c * BLK:(c + 1) * BLK] = o[:, CI_N:2 * CI_N].T.ravel()
        pab += o[:, 2 * CI_N:3 * CI_N].sum()

    sx = aux["sx"]
    sy = aux["sy"]
    # closed-form squared-distance Frobenius norms, consistent with the
    # device's mixed n_i/n_j paths (true zero diag):
    sq_a = n * (aux["sum_nbx"] + aux["sum_nx"]) - 2.0 * np.dot(sx, sx)
    sq_b = n * (aux["sum_nby"] + aux["sum_ny"]) - 2.0 * np.dot(sy, sy)

    sa = rs_a - MU          # true (zero-diag) row sums of a
    sb = rs_b - MU
    sat = sa - n * MU       # row sums of (a - mu)
    sbt = sb - n * MU
    Ua = sat.sum()
    Ub = sbt.sum()
    # device pab = sum (a-mu)*b (diag contributes ~0 in device and truth)
    Sab = pab - MU * (sa.sum() - MU * n * n)
    Saa = sq_a - 2.0 * MU * sa.sum() + MU2 * n * n
    Sbb = sq_b - 2.0 * MU * sb.sum() + MU2 * n * n

    sumAB = Sab - 2.0 * np.dot(sat, sbt) / n + Ua * Ub / n**2
    sumAA = Saa - 2.0 * np.dot(sat, sat) / n + Ua * Ua / n**2
    sumBB = Sbb - 2.0 * np.dot(sbt, sbt) / n + Ub * Ub / n**2

    inv_n2 = 1.0 / (n * n)
    dcov2_xy = sumAB * inv_n2
    dcov2_xx = sumAA * inv_n2
    dcov2_yy = sumBB * inv_n2
    dcor = -np.sqrt(dcov2_xy) / np.sqrt(np.sqrt(dcov2_xx) * np.sqrt(dcov2_yy))
    return np.asarray(dcor, dtype=np.float32)


def run(x, y, mm_mode=None, trace=False, tmpdir=None):
    nc = _get_program()
    in_maps, aux = make_in_maps(x, y)
    res = run_bass_kernel_spmd(nc, in_maps, core_ids=list(range(NCORES)),
                               trace=trace, tmpdir=tmpdir)
    outs = [r["out"] for r in res.results]
    return finalize(outs, aux), res


def kernel(x, y):
    val, _ = run(x, y)
    return val
